# revision 18
# baseline (speedup 1.0000x reference)
"""Trainium2 Bass kernel for nn_DecoderTransformer (T=2048, D=2048, H=16, V=32000).

v3 strategy (8-way tensor parallel, full inputs in / full output out):
  - All matmuls bf16 (inputs pre-cast on host), f32 psum accumulation.
  - Fully interleaved prefix: per 512-token block b, gather x = we[tok]+pe
    (bf16), XBAR DMA-transpose it into a persistent xT [D, T] tile (no PE
    transposes at all), project q/k ([hs, t]) and v ([t, hs]) for the core's
    2 heads, run causal attention for block b, compute the proj partial for
    those 512 columns (contraction over the 2 local heads, +proj_b/8), and
    issue a bf16 AllReduce per quarter.  AR(q0) completes while later blocks
    still compute.
  - resid built in place per quarter: xT[:, q] += AR output.
  - fc sharded over vocab (4000 cols/core), fw streamed as the moving operand
    in natural layout; out[t, voc] written bf16 and upcast on the host.
    Half-major (t 0:1024 then 1024:2048) so only AR0/AR1 gate the start.
"""

import os

import numpy as np

T = 2048
D = 2048
H = 16
HS = 128
V = 32000
NCORES = 8
P = 128
DC = D // P            # 16 d chunks
TC = T // P            # 16 t chunks
NTB = T // 512         # 4 t-blocks of 512
HPC = H // NCORES      # 2 heads per core
VSH = V // NCORES      # 4000 vocab shard
VCW = 500              # vocab chunk width (4000 = 8*500)
NVC = VSH // VCW       # 8 vocab chunks

_CACHE = {}


def _build():
    import concourse.bass as bass
    import concourse.tile as tile
    from concourse import bacc, mybir

    f32 = mybir.dt.float32
    bf16 = mybir.dt.bfloat16
    i32 = mybir.dt.int32
    EXP = mybir.ActivationFunctionType.Exp
    IDENT = mybir.ActivationFunctionType.Identity
    RG = [list(range(NCORES))]

    nc = bacc.Bacc("TRN2", target_bir_lowering=False, debug=False,
                   num_devices=NCORES, num_swdge_queues=4)

    tok = nc.dram_tensor("tok", [T], i32, kind="ExternalInput")
    web = nc.dram_tensor("web", [V, D], bf16, kind="ExternalInput")
    peb = nc.dram_tensor("peb", [T, D], bf16, kind="ExternalInput")
    wqb = nc.dram_tensor("wqb", [D, HPC * HS], bf16, kind="ExternalInput")
    wkb = nc.dram_tensor("wkb", [D, HPC * HS], bf16, kind="ExternalInput")
    wvb = nc.dram_tensor("wvb", [D, HPC * HS], bf16, kind="ExternalInput")
    pwb = nc.dram_tensor("pwb", [HPC * HS, D], bf16, kind="ExternalInput")
    pb8 = nc.dram_tensor("pb8", [D, 1], f32, kind="ExternalInput")
    fwb = nc.dram_tensor("fwb", [D, VSH], bf16, kind="ExternalInput")
    fbv = nc.dram_tensor("fbv", [1, VSH], f32, kind="ExternalInput")
    out_d = nc.dram_tensor("out", [T, VSH], bf16, kind="ExternalOutput")

    with nc.allow_low_precision(reason="bf16 decoder kernel"), \
         tile.TileContext(nc) as tc:
        dram = tc.alloc_tile_pool(name="dram", bufs=1, space="DRAM")
        pconst = tc.alloc_tile_pool(name="pconst", bufs=1)
        pxT = tc.alloc_tile_pool(name="pxT", bufs=1)

        ones_col = pconst.tile([P, 1], bf16, name="ones_col")
        nc.vector.memset(ones_col[:], 1.0)
        ones_row = pconst.tile([1, P], bf16, name="ones_row")
        nc.vector.memset(ones_row[:], 1.0)
        pb8_s = pconst.tile([P, DC], f32, name="pb8_s")
        nc.sync.dma_start(out=pb8_s[:],
                          in_=pb8[:].rearrange("(dc p) one -> p (dc one)", p=P))
        # additive causal mask, shifted views: maskadd_m[s, t] with
        # m in 0..3 = maskbig[:, 384-128m : 896-128m].
        maskbig = pconst.tile([P, 896], f32, name="maskbig")
        nc.gpsimd.memset(maskbig[:], 0.0)
        nc.gpsimd.affine_select(
            out=maskbig[:], in_=maskbig[:],
            compare_op=mybir.AluOpType.is_ge, fill=-40.0,
            base=-384, pattern=[[1, 896]], channel_multiplier=-1,
        )

        xT = pxT.tile([P, DC, T], bf16, name="xT")

        partial_d = [dram.tile([D, 512], bf16, name=f"partial{q}")
                     for q in range(NTB)]
        resid_q = [dram.tile([D, 512], bf16, name=f"resid{q}",
                             addr_space="Shared") for q in range(NTB)]

        with tc.tile_pool(name="pqa", bufs=1) as pqa, \
             tc.tile_pool(name="pemb", bufs=1) as pemb, \
             tc.tile_pool(name="patt", bufs=1) as patt, \
             tc.tile_pool(name="ps_qk", bufs=2, space="PSUM") as ps_qk, \
             tc.tile_pool(name="ps_v", bufs=1, space="PSUM") as ps_v, \
             tc.tile_pool(name="ps_sm", bufs=2, space="PSUM") as ps_sm, \
             tc.tile_pool(name="ps_cs", bufs=1, space="PSUM") as ps_cs, \
             tc.tile_pool(name="ps_av", bufs=1, space="PSUM") as ps_av, \
             tc.tile_pool(name="ps_bc", bufs=1, space="PSUM") as ps_bc:
            kT = pqa.tile([P, HPC, T], bf16, name="kT")
            v_both = pqa.tile([P, TC, HPC * HS], bf16, name="v_both")
            wq_s = pqa.tile([P, DC, HPC * HS], bf16, name="wq_s")
            wk_s = pqa.tile([P, DC, HPC * HS], bf16, name="wk_s")
            wv_s = pqa.tile([P, DC, HPC * HS], bf16, name="wv_s")
            pw_s = pqa.tile([P, HPC, D], bf16, name="pw_s")
            for wdram, wt in ((wqb, wq_s), (wkb, wk_s), (wvb, wv_s)):
                nc.sync.dma_start(
                    out=wt[:],
                    in_=wdram[:].rearrange("(dc p) j -> p dc j", p=P))
            nc.sync.dma_start(
                out=pw_s[:], in_=pwb[:].rearrange("(c p) d -> p c d", p=P))

            for b in range(NTB):
                # ---- gather + pe add + XBAR transpose, 4 t-chunks ----
                for k in range(4):
                    tcc = b * 4 + k
                    idx_t = pemb.tile([P, 1], i32, tag="idx", bufs=6,
                                      name=f"idx{tcc}")
                    nc.sync.dma_start(
                        out=idx_t[:],
                        in_=tok[:][tcc * P:(tcc + 1) * P, None])
                    xg = pemb.tile([P, D], bf16, tag="xg", bufs=5,
                                   name=f"xg{tcc}")
                    gi = nc.gpsimd.indirect_dma_start(
                        out=xg[:], out_offset=None, in_=web[:],
                        in_offset=bass.IndirectOffsetOnAxis(
                            ap=idx_t[:, :1], axis=0))
                    if tcc % 4:
                        gi.ins.queue = f"qPoolDynamic{tcc % 4}"
                    pet = pemb.tile([P, D], bf16, tag="pet", bufs=4,
                                    name=f"pet{tcc}")
                    nc.scalar.dma_start(
                        out=pet[:], in_=peb[:][tcc * P:(tcc + 1) * P, :])
                    nc.vector.tensor_add(xg[:], xg[:], pet[:])
                    nc.sync.dma_start_transpose(
                        out=xT[:, :, tcc * P:(tcc + 1) * P], in_=xg[:])
                # ---- q, k for block b ([hs, t]); v in [t, hs] ----
                qTb = patt.tile([P, HPC, 512], bf16, tag="qTb", bufs=2,
                                name=f"qTb{b}")
                for wt, dst in ((wq_s, qTb), (wk_s, None)):
                    for h in range(HPC):
                        ps = ps_qk.tile([P, 512], f32, tag="qkps",
                                        name=f"qk{wt.name}{h}_{b}")
                        for dc in range(DC):
                            nc.tensor.matmul(
                                ps[:], wt[:, dc, h * HS:(h + 1) * HS],
                                xT[:, dc, b * 512:(b + 1) * 512],
                                start=(dc == 0), stop=(dc == DC - 1))
                        if dst is None:
                            nc.vector.tensor_copy(
                                kT[:, h, b * 512:(b + 1) * 512], ps[:])
                        else:
                            nc.vector.tensor_copy(dst[:, h, :], ps[:])
                for tq in range(4):
                    tcc = b * 4 + tq
                    psv = ps_v.tile([P, HPC * HS], f32, tag="vps",
                                    name=f"psv{tcc}")
                    for dc in range(DC):
                        nc.tensor.matmul(
                            psv[:], xT[:, dc, tcc * P:(tcc + 1) * P],
                            wv_s[:, dc, :],
                            start=(dc == 0), stop=(dc == DC - 1))
                    nc.vector.tensor_copy(v_both[:, tcc, :], psv[:])

                # ---- causal attention for block g = b ----
                g = b
                headsb = patt.tile([P, HPC, 512], bf16, tag="headsb", bufs=2,
                                   name=f"headsb{g}")
                for h in range(HPC):
                    nsc = 4 * g + 4
                    expT = patt.tile([P, TC, 512], bf16, tag="expT",
                                     name=f"expT{h}_{g}")
                    cs_ps = ps_cs.tile([1, 512], f32, tag="cs",
                                       name=f"cs{h}_{g}")
                    for c in range(nsc):
                        s_ps = ps_sm.tile([P, 512], f32, tag="sims",
                                          name=f"sims{h}_{g}_{c}")
                        nc.tensor.matmul(
                            s_ps[:], kT[:, h, c * P:(c + 1) * P],
                            qTb[:, h, :], start=True, stop=True)
                        if c >= 4 * g:
                            m = c - 4 * g
                            nc.vector.tensor_add(
                                s_ps[:], s_ps[:],
                                maskbig[:, 384 - 128 * m:896 - 128 * m])
                        nc.scalar.activation(out=expT[:, c, :],
                                             in_=s_ps[:], func=EXP)
                    for c in range(nsc):
                        nc.tensor.matmul(cs_ps[:], ones_col[:],
                                         expT[:, c, :],
                                         start=(c == 0), stop=(c == nsc - 1))
                    av_ps = ps_av.tile([P, 512], f32, tag="av",
                                       name=f"av{h}_{g}")
                    for c in range(nsc):
                        nc.tensor.matmul(
                            av_ps[:], v_both[:, c, h * HS:(h + 1) * HS],
                            expT[:, c, :],
                            start=(c == 0), stop=(c == nsc - 1))
                    recip = patt.tile([1, 512], f32, tag="recip",
                                      bufs=2, name=f"rc{h}_{g}")
                    nc.vector.reciprocal(recip[:], cs_ps[:])
                    recb = patt.tile([1, 512], bf16, tag="recb",
                                     bufs=2, name=f"rcb{h}_{g}")
                    nc.vector.tensor_copy(recb[:], recip[:])
                    bc_ps = ps_bc.tile([P, 512], f32, tag="bc",
                                       name=f"bc{h}_{g}")
                    nc.tensor.matmul(bc_ps[:], ones_row[:], recb[:],
                                     start=True, stop=True)
                    bc_s = patt.tile([P, 512], f32, tag="bc_s",
                                     bufs=2, name=f"bcs{h}_{g}")
                    nc.vector.tensor_copy(bc_s[:], bc_ps[:])
                    nc.vector.tensor_mul(headsb[:, h, :], av_ps[:], bc_s[:])

                # ---- proj partial for quarter b (+pb/8), then AllReduce ----
                psb = patt.tile([P, DC, 512], bf16, tag="psb",
                                name=f"psb{b}")
                for do in range(DC):
                    pp = ps_sm.tile([P, 512], f32, tag="sims",
                                    name=f"pp{b}_{do}")
                    for c in range(HPC):
                        nc.tensor.matmul(
                            pp[:], pw_s[:, c, do * P:(do + 1) * P],
                            headsb[:, c, :],
                            start=(c == 0), stop=(c == HPC - 1))
                    nc.scalar.activation(out=psb[:, do, :], in_=pp[:],
                                         func=IDENT,
                                         bias=pb8_s[:, do:do + 1])
                nc.scalar.dma_start(
                    out=partial_d[b][:].rearrange("(dc p) t -> p dc t", p=P),
                    in_=psb[:])
                nc.gpsimd.collective_compute(
                    "AllReduce", mybir.AluOpType.add, replica_groups=RG,
                    ins=[partial_d[b][:]], outs=[resid_q[b][:]])

        # ---- fc phase: resid in place, then out = resid @ fw + fb ----
        with tc.tile_pool(name="pfc", bufs=1) as pfc, \
             tc.tile_pool(name="ps_fc", bufs=6, space="PSUM") as ps_fc:
            fb_sb = pfc.tile([1, VSH], f32, name="fb_sb")
            nc.scalar.dma_start(out=fb_sb[:], in_=fbv[:])
            fb_bf = pfc.tile([1, VSH], bf16, name="fb_bf")
            nc.vector.tensor_copy(fb_bf[:], fb_sb[:])
            fb_bcast = pfc.tile([P, VSH], f32, name="fb_bcast")
            for vc in range(NVC):
                fps = ps_fc.tile([P, VCW], f32, tag="fcps", name=f"fbb{vc}")
                nc.tensor.matmul(fps[:], ones_row[:],
                                 fb_bf[:, vc * VCW:(vc + 1) * VCW],
                                 start=True, stop=True)
                nc.vector.tensor_copy(fb_bcast[:, vc * VCW:(vc + 1) * VCW],
                                      fps[:])

            def build_resid(q, eng):
                ast = pfc.tile([P, DC, 512], bf16, tag="arst", bufs=2,
                               name=f"arst{q}")
                eng.dma_start(
                    out=ast[:],
                    in_=resid_q[q][:].rearrange("(dc p) t -> p dc t", p=P))
                nc.vector.tensor_add(
                    xT[:, :, q * 512:(q + 1) * 512],
                    xT[:, :, q * 512:(q + 1) * 512], ast[:])

            def load_fw(vc, nm, eng):
                t = pfc.tile([P, DC, VCW], bf16, tag="fw_t", bufs=3, name=nm)
                eng.dma_start(
                    out=t[:],
                    in_=fwb[:][:, vc * VCW:(vc + 1) * VCW]
                    .rearrange("(dc p) v -> p dc v", p=P))
                return t

            def fc_pass(vc, half, fw_t, tag):
                for tc8 in range(8):
                    toff = half * 1024 + tc8 * P
                    ps = ps_fc.tile([P, VCW], f32, tag="fcps",
                                    name=f"fc{tag}_{vc}_{half}_{tc8}")
                    for dc in range(DC):
                        nc.tensor.matmul(
                            ps[:], xT[:, dc, toff:toff + P],
                            fw_t[:, dc, :],
                            start=(dc == 0), stop=(dc == DC - 1))
                    ev = pfc.tile([P, VCW], bf16, tag="fc_ev", bufs=4,
                                  name=f"fcev{tag}_{vc}_{half}_{tc8}")
                    nc.vector.tensor_add(
                        ev[:], ps[:], fb_bcast[:, vc * VCW:(vc + 1) * VCW])
                    nc.scalar.dma_start(
                        out=out_d[:][toff:toff + P,
                                     vc * VCW:(vc + 1) * VCW],
                        in_=ev[:])

            # sync prefetches fw0-2 ahead of the AR-gated resid reloads so
            # the queue never blocks fc's weight stream on a collective;
            # q0/q1 reload in parallel on sync/scalar, q2/q3 on gpsimd.
            fwt = {}
            for vc in range(3):
                fwt[vc] = load_fw(vc, f"fw{vc}a", nc.sync)
            build_resid(0, nc.sync)
            build_resid(1, nc.scalar)
            for vc in range(3, NVC):
                fwt[vc] = load_fw(vc, f"fw{vc}a", nc.sync)
            build_resid(2, nc.gpsimd)
            build_resid(3, nc.gpsimd)
            for vc in range(NVC):
                fc_pass(vc, 0, fwt[vc], "a")
            for vc in range(NVC):
                fc_pass(vc, 1, load_fw(vc, f"fw{vc}b",
                                       nc.gpsimd if vc % 2 else nc.sync), "b")

        pxT.release()
        pconst.release()
        dram.release()

    if not int(os.environ.get("BASSKERNEL_SKIP_COMPILE", "0")):
        nc.compile()
    return nc


def _get_nc():
    if "nc" not in _CACHE:
        _CACHE["nc"] = _build()
    return _CACHE["nc"]


def kernel(token_ids, we, pe, Wq, Wk, Wv, proj_w, proj_b, fc_w, fc_b):
    import ml_dtypes

    from concourse.bass_utils import run_bass_kernel_spmd

    bf16 = ml_dtypes.bfloat16

    tok = np.asarray(token_ids).astype(np.int32)
    web = np.ascontiguousarray(np.asarray(we)).astype(bf16)
    peb = np.ascontiguousarray(np.asarray(pe))[:T].astype(bf16)
    Wq = np.asarray(Wq, dtype=np.float32)
    Wk = np.asarray(Wk, dtype=np.float32)
    Wv = np.asarray(Wv, dtype=np.float32)
    proj_w = np.asarray(proj_w, dtype=np.float32)
    proj_b = np.asarray(proj_b, dtype=np.float32)
    fc_w = np.asarray(fc_w, dtype=np.float32)
    fc_b = np.asarray(fc_b, dtype=np.float32)

    scale = np.float32(1.0 / np.sqrt(HS))
    pb8 = (proj_b / NCORES).reshape(D, 1).astype(np.float32)
    in_maps = []
    for i in range(NCORES):
        h0 = HPC * i
        wq_i = np.ascontiguousarray(
            np.concatenate([Wq[h0 + j] for j in range(HPC)], axis=1)) * scale
        wk_i = np.ascontiguousarray(
            np.concatenate([Wk[h0 + j] for j in range(HPC)], axis=1))
        wv_i = np.ascontiguousarray(
            np.concatenate([Wv[h0 + j] for j in range(HPC)], axis=1))
        pw_i = np.ascontiguousarray(
            proj_w[HPC * HS * i:HPC * HS * (i + 1), :])
        fw_i = np.ascontiguousarray(fc_w[:, VSH * i:VSH * (i + 1)])
        fb_i = np.ascontiguousarray(
            fc_b[VSH * i:VSH * (i + 1)].reshape(1, VSH)).astype(np.float32)
        in_maps.append({
            "tok": tok, "web": web, "peb": peb,
            "wqb": wq_i.astype(bf16), "wkb": wk_i.astype(bf16),
            "wvb": wv_i.astype(bf16),
            "pwb": pw_i.astype(bf16), "pb8": pb8,
            "fwb": fw_i.astype(bf16), "fbv": fb_i,
        })

    nc = _get_nc()
    trace = bool(int(os.environ.get("BASSKERNEL_TRACE", "0")))
    res = run_bass_kernel_spmd(nc, in_maps, core_ids=list(range(NCORES)),
                               trace=trace)
    if trace and res.exec_time_ns is not None:
        print(f"HW exec time: {res.exec_time_ns} ns")
        if res.instructions_and_trace is not None:
            print(f"Trace: {res.instructions_and_trace[1]}")

    out = np.empty((T, V), dtype=np.float32)
    for i in range(NCORES):
        out[:, VSH * i:VSH * (i + 1)] = res.results[i]["out"].astype(
            np.float32)
    return out


# revision 20
# speedup vs baseline: 1.0140x; 1.0140x over previous
"""Trainium2 Bass kernel for nn_DecoderTransformer (T=2048, D=2048, H=16, V=32000).

v3 strategy (8-way tensor parallel, full inputs in / full output out):
  - All matmuls bf16 (inputs pre-cast on host), f32 psum accumulation.
  - Fully interleaved prefix: per 512-token block b, gather x = we[tok]+pe
    (bf16), XBAR DMA-transpose it into a persistent xT [D, T] tile (no PE
    transposes at all), project q/k ([hs, t]) and v ([t, hs]) for the core's
    2 heads, run causal attention for block b, compute the proj partial for
    those 512 columns (contraction over the 2 local heads, +proj_b/8), and
    issue a bf16 AllReduce per quarter.  AR(q0) completes while later blocks
    still compute.
  - resid built in place per quarter: xT[:, q] += AR output.
  - fc sharded over vocab (4000 cols/core), fw streamed as the moving operand
    in natural layout; out[t, voc] written bf16 and upcast on the host.
    Half-major (t 0:1024 then 1024:2048) so only AR0/AR1 gate the start.
"""

import os

import numpy as np

T = 2048
D = 2048
H = 16
HS = 128
V = 32000
NCORES = 8
P = 128
DC = D // P            # 16 d chunks
TC = T // P            # 16 t chunks
NTB = T // 512         # 4 t-blocks of 512
HPC = H // NCORES      # 2 heads per core
VSH = V // NCORES      # 4000 vocab shard
VCW = 500              # vocab chunk width (4000 = 8*500)
NVC = VSH // VCW       # 8 vocab chunks

_CACHE = {}


def _build():
    import concourse.bass as bass
    import concourse.tile as tile
    from concourse import bacc, mybir

    f32 = mybir.dt.float32
    bf16 = mybir.dt.bfloat16
    i32 = mybir.dt.int32
    EXP = mybir.ActivationFunctionType.Exp
    IDENT = mybir.ActivationFunctionType.Identity
    RG = [list(range(NCORES))]

    nc = bacc.Bacc("TRN2", target_bir_lowering=False, debug=False,
                   num_devices=NCORES, num_swdge_queues=4)

    tok = nc.dram_tensor("tok", [T], i32, kind="ExternalInput")
    web = nc.dram_tensor("web", [V, D], bf16, kind="ExternalInput")
    peb = nc.dram_tensor("peb", [T, D], bf16, kind="ExternalInput")
    wqb = nc.dram_tensor("wqb", [D, HPC * HS], bf16, kind="ExternalInput")
    wkb = nc.dram_tensor("wkb", [D, HPC * HS], bf16, kind="ExternalInput")
    wvb = nc.dram_tensor("wvb", [D, HPC * HS], bf16, kind="ExternalInput")
    pwb = nc.dram_tensor("pwb", [HPC * HS, D], bf16, kind="ExternalInput")
    pb8 = nc.dram_tensor("pb8", [D, 1], f32, kind="ExternalInput")
    fwb = nc.dram_tensor("fwb", [D, VSH], bf16, kind="ExternalInput")
    fbv = nc.dram_tensor("fbv", [1, VSH], f32, kind="ExternalInput")
    out_d = nc.dram_tensor("out", [T, VSH], bf16, kind="ExternalOutput")

    with nc.allow_low_precision(reason="bf16 decoder kernel"), \
         tile.TileContext(nc) as tc:
        dram = tc.alloc_tile_pool(name="dram", bufs=1, space="DRAM")
        pconst = tc.alloc_tile_pool(name="pconst", bufs=1)
        pxT = tc.alloc_tile_pool(name="pxT", bufs=1)
        pfw = tc.alloc_tile_pool(name="pfw", bufs=1)

        ones_col = pconst.tile([P, 1], bf16, name="ones_col")
        nc.vector.memset(ones_col[:], 1.0)
        ones_row = pconst.tile([1, P], bf16, name="ones_row")
        nc.vector.memset(ones_row[:], 1.0)
        pb8_s = pconst.tile([P, DC], f32, name="pb8_s")
        nc.sync.dma_start(out=pb8_s[:],
                          in_=pb8[:].rearrange("(dc p) one -> p (dc one)", p=P))
        # additive causal mask, shifted views: maskadd_m[s, t] with
        # m in 0..3 = maskbig[:, 384-128m : 896-128m].
        maskbig = pconst.tile([P, 896], f32, name="maskbig")
        nc.gpsimd.memset(maskbig[:], 0.0)
        nc.gpsimd.affine_select(
            out=maskbig[:], in_=maskbig[:],
            compare_op=mybir.AluOpType.is_ge, fill=-40.0,
            base=-384, pattern=[[1, 896]], channel_multiplier=-1,
        )

        xT = pxT.tile([P, DC, T], bf16, name="xT")
        fw0_t = pfw.tile([P, DC, VCW], bf16, name="fw0p")

        partial_d = [dram.tile([D, 512], bf16, name=f"partial{q}")
                     for q in range(NTB)]
        resid_q = [dram.tile([D, 512], bf16, name=f"resid{q}",
                             addr_space="Shared") for q in range(NTB)]

        with tc.tile_pool(name="pqa", bufs=1) as pqa, \
             tc.tile_pool(name="pemb", bufs=1) as pemb, \
             tc.tile_pool(name="patt", bufs=1) as patt, \
             tc.tile_pool(name="ps_qk", bufs=2, space="PSUM") as ps_qk, \
             tc.tile_pool(name="ps_v", bufs=1, space="PSUM") as ps_v, \
             tc.tile_pool(name="ps_sm", bufs=2, space="PSUM") as ps_sm, \
             tc.tile_pool(name="ps_cs", bufs=1, space="PSUM") as ps_cs, \
             tc.tile_pool(name="ps_av", bufs=1, space="PSUM") as ps_av, \
             tc.tile_pool(name="ps_bc", bufs=1, space="PSUM") as ps_bc:
            kT = pqa.tile([P, HPC, T], bf16, name="kT")
            v_both = pqa.tile([P, TC, HPC * HS], bf16, name="v_both")
            wq_s = pqa.tile([P, DC, HPC * HS], bf16, name="wq_s")
            wk_s = pqa.tile([P, DC, HPC * HS], bf16, name="wk_s")
            wv_s = pqa.tile([P, DC, HPC * HS], bf16, name="wv_s")
            pw_s = pqa.tile([P, HPC, D], bf16, name="pw_s")
            for wdram, wt in ((wqb, wq_s), (wkb, wk_s), (wvb, wv_s)):
                nc.sync.dma_start(
                    out=wt[:],
                    in_=wdram[:].rearrange("(dc p) j -> p dc j", p=P))
            nc.sync.dma_start(
                out=pw_s[:], in_=pwb[:].rearrange("(c p) d -> p c d", p=P))

            for b in range(NTB):
                # ---- gather + pe add + XBAR transpose, 4 t-chunks ----
                for k in range(4):
                    tcc = b * 4 + k
                    idx_t = pemb.tile([P, 1], i32, tag="idx", bufs=3,
                                      name=f"idx{tcc}")
                    nc.sync.dma_start(
                        out=idx_t[:],
                        in_=tok[:][tcc * P:(tcc + 1) * P, None])
                    xg = pemb.tile([P, D], bf16, tag="xg", bufs=4,
                                   name=f"xg{tcc}")
                    gi = nc.gpsimd.indirect_dma_start(
                        out=xg[:], out_offset=None, in_=web[:],
                        in_offset=bass.IndirectOffsetOnAxis(
                            ap=idx_t[:, :1], axis=0))
                    if tcc % 4:
                        gi.ins.queue = f"qPoolDynamic{tcc % 4}"
                    pet = pemb.tile([P, D], bf16, tag="pet", bufs=3,
                                    name=f"pet{tcc}")
                    nc.scalar.dma_start(
                        out=pet[:], in_=peb[:][tcc * P:(tcc + 1) * P, :])
                    nc.vector.tensor_add(xg[:], xg[:], pet[:])
                    nc.sync.dma_start_transpose(
                        out=xT[:, :, tcc * P:(tcc + 1) * P], in_=xg[:])
                # ---- q, k for block b ([hs, t]); v in [t, hs] ----
                qTb = patt.tile([P, HPC, 512], bf16, tag="qTb", bufs=1,
                                name=f"qTb{b}")
                for wt, dst in ((wq_s, qTb), (wk_s, None)):
                    for h in range(HPC):
                        ps = ps_qk.tile([P, 512], f32, tag="qkps",
                                        name=f"qk{wt.name}{h}_{b}")
                        for dc in range(DC):
                            nc.tensor.matmul(
                                ps[:], wt[:, dc, h * HS:(h + 1) * HS],
                                xT[:, dc, b * 512:(b + 1) * 512],
                                start=(dc == 0), stop=(dc == DC - 1))
                        if dst is None:
                            nc.vector.tensor_copy(
                                kT[:, h, b * 512:(b + 1) * 512], ps[:])
                        else:
                            nc.vector.tensor_copy(dst[:, h, :], ps[:])
                for tq in range(4):
                    tcc = b * 4 + tq
                    psv = ps_v.tile([P, HPC * HS], f32, tag="vps",
                                    name=f"psv{tcc}")
                    for dc in range(DC):
                        nc.tensor.matmul(
                            psv[:], xT[:, dc, tcc * P:(tcc + 1) * P],
                            wv_s[:, dc, :],
                            start=(dc == 0), stop=(dc == DC - 1))
                    nc.vector.tensor_copy(v_both[:, tcc, :], psv[:])

                if b == 2:
                    # prefetch fc's first weight tile into the permanent
                    # pool - no pool-transition space gate, so it overlaps
                    # the prefix instead of waiting for attention to drain.
                    nc.sync.dma_start(
                        out=fw0_t[:],
                        in_=fwb[:][:, 0:VCW]
                        .rearrange("(dc p) v -> p dc v", p=P))
                # ---- causal attention for block g = b ----
                g = b
                headsb = patt.tile([P, HPC, 512], bf16, tag="headsb", bufs=2,
                                   name=f"headsb{g}")
                for h in range(HPC):
                    nsc = 4 * g + 4
                    expT = patt.tile([P, TC, 512], bf16, tag="expT",
                                     name=f"expT{h}_{g}")
                    cs_ps = ps_cs.tile([1, 512], f32, tag="cs",
                                       name=f"cs{h}_{g}")
                    for c in range(nsc):
                        s_ps = ps_sm.tile([P, 512], f32, tag="sims",
                                          name=f"sims{h}_{g}_{c}")
                        nc.tensor.matmul(
                            s_ps[:], kT[:, h, c * P:(c + 1) * P],
                            qTb[:, h, :], start=True, stop=True)
                        if c >= 4 * g:
                            m = c - 4 * g
                            nc.vector.tensor_add(
                                s_ps[:], s_ps[:],
                                maskbig[:, 384 - 128 * m:896 - 128 * m])
                        nc.scalar.activation(out=expT[:, c, :],
                                             in_=s_ps[:], func=EXP)
                    for c in range(nsc):
                        nc.tensor.matmul(cs_ps[:], ones_col[:],
                                         expT[:, c, :],
                                         start=(c == 0), stop=(c == nsc - 1))
                    av_ps = ps_av.tile([P, 512], f32, tag="av",
                                       name=f"av{h}_{g}")
                    for c in range(nsc):
                        nc.tensor.matmul(
                            av_ps[:], v_both[:, c, h * HS:(h + 1) * HS],
                            expT[:, c, :],
                            start=(c == 0), stop=(c == nsc - 1))
                    recip = patt.tile([1, 512], f32, tag="recip",
                                      bufs=1, name=f"rc{h}_{g}")
                    nc.vector.reciprocal(recip[:], cs_ps[:])
                    recb = patt.tile([1, 512], bf16, tag="recb",
                                     bufs=1, name=f"rcb{h}_{g}")
                    nc.vector.tensor_copy(recb[:], recip[:])
                    bc_ps = ps_bc.tile([P, 512], f32, tag="bc",
                                       name=f"bc{h}_{g}")
                    nc.tensor.matmul(bc_ps[:], ones_row[:], recb[:],
                                     start=True, stop=True)
                    bc_s = patt.tile([P, 512], f32, tag="bc_s",
                                     bufs=1, name=f"bcs{h}_{g}")
                    nc.vector.tensor_copy(bc_s[:], bc_ps[:])
                    nc.vector.tensor_mul(headsb[:, h, :], av_ps[:], bc_s[:])

                # ---- proj partial for quarter b (+pb/8), then AllReduce ----
                psb = patt.tile([P, DC, 512], bf16, tag="psb",
                                name=f"psb{b}")
                for do in range(DC):
                    pp = ps_sm.tile([P, 512], f32, tag="sims",
                                    name=f"pp{b}_{do}")
                    for c in range(HPC):
                        nc.tensor.matmul(
                            pp[:], pw_s[:, c, do * P:(do + 1) * P],
                            headsb[:, c, :],
                            start=(c == 0), stop=(c == HPC - 1))
                    nc.scalar.activation(out=psb[:, do, :], in_=pp[:],
                                         func=IDENT,
                                         bias=pb8_s[:, do:do + 1])
                nc.scalar.dma_start(
                    out=partial_d[b][:].rearrange("(dc p) t -> p dc t", p=P),
                    in_=psb[:])
                nc.gpsimd.collective_compute(
                    "AllReduce", mybir.AluOpType.add, replica_groups=RG,
                    ins=[partial_d[b][:]], outs=[resid_q[b][:]])

        # ---- fc phase: resid in place, then out = resid @ fw + fb ----
        with tc.tile_pool(name="pfc", bufs=1) as pfc, \
             tc.tile_pool(name="ps_fc", bufs=6, space="PSUM") as ps_fc:
            fb_sb = pfc.tile([1, VSH], f32, name="fb_sb")
            nc.scalar.dma_start(out=fb_sb[:], in_=fbv[:])
            fb_bf = pfc.tile([1, VSH], bf16, name="fb_bf")
            nc.vector.tensor_copy(fb_bf[:], fb_sb[:])
            fb_bcast = pfc.tile([P, VSH], f32, name="fb_bcast")
            for vc in range(NVC):
                fps = ps_fc.tile([P, VCW], f32, tag="fcps", name=f"fbb{vc}")
                nc.tensor.matmul(fps[:], ones_row[:],
                                 fb_bf[:, vc * VCW:(vc + 1) * VCW],
                                 start=True, stop=True)
                nc.vector.tensor_copy(fb_bcast[:, vc * VCW:(vc + 1) * VCW],
                                      fps[:])

            def build_resid(q, eng):
                ast = pfc.tile([P, DC, 512], bf16, tag="arst", bufs=2,
                               name=f"arst{q}")
                eng.dma_start(
                    out=ast[:],
                    in_=resid_q[q][:].rearrange("(dc p) t -> p dc t", p=P))
                nc.vector.tensor_add(
                    xT[:, :, q * 512:(q + 1) * 512],
                    xT[:, :, q * 512:(q + 1) * 512], ast[:])

            def load_fw(vc, nm, eng):
                t = pfc.tile([P, DC, VCW], bf16, tag="fw_t", bufs=3, name=nm)
                eng.dma_start(
                    out=t[:],
                    in_=fwb[:][:, vc * VCW:(vc + 1) * VCW]
                    .rearrange("(dc p) v -> p dc v", p=P))
                return t

            def fc_pass(vc, half, fw_t, tag):
                for tc8 in range(8):
                    toff = half * 1024 + tc8 * P
                    ps = ps_fc.tile([P, VCW], f32, tag="fcps",
                                    name=f"fc{tag}_{vc}_{half}_{tc8}")
                    for dc in range(DC):
                        nc.tensor.matmul(
                            ps[:], xT[:, dc, toff:toff + P],
                            fw_t[:, dc, :],
                            start=(dc == 0), stop=(dc == DC - 1))
                    ev = pfc.tile([P, VCW], bf16, tag="fc_ev", bufs=4,
                                  name=f"fcev{tag}_{vc}_{half}_{tc8}")
                    nc.vector.tensor_add(
                        ev[:], ps[:], fb_bcast[:, vc * VCW:(vc + 1) * VCW])
                    nc.scalar.dma_start(
                        out=out_d[:][toff:toff + P,
                                     vc * VCW:(vc + 1) * VCW],
                        in_=ev[:])

            # sync stream: fw0, arst0, fw1, arst1, fw2.. ; gpsimd: arst2/3
            fwt = {0: fw0_t}
            fwt[1] = load_fw(1, "fw1a", nc.sync)
            build_resid(0, nc.sync)
            build_resid(1, nc.scalar)
            for vc in range(2, NVC):
                fwt[vc] = load_fw(vc, f"fw{vc}a", nc.sync)
            build_resid(2, nc.gpsimd)
            build_resid(3, nc.gpsimd)
            for vc in range(NVC):
                fc_pass(vc, 0, fwt[vc], "a")
            for vc in range(NVC):
                fc_pass(vc, 1, load_fw(vc, f"fw{vc}b",
                                       nc.gpsimd if vc % 2 else nc.sync), "b")

        pfw.release()
        pxT.release()
        pconst.release()
        dram.release()

    if not int(os.environ.get("BASSKERNEL_SKIP_COMPILE", "0")):
        nc.compile()
    return nc


def _get_nc():
    if "nc" not in _CACHE:
        _CACHE["nc"] = _build()
    return _CACHE["nc"]


def kernel(token_ids, we, pe, Wq, Wk, Wv, proj_w, proj_b, fc_w, fc_b):
    import ml_dtypes

    from concourse.bass_utils import run_bass_kernel_spmd

    bf16 = ml_dtypes.bfloat16

    tok = np.asarray(token_ids).astype(np.int32)
    web = np.ascontiguousarray(np.asarray(we)).astype(bf16)
    peb = np.ascontiguousarray(np.asarray(pe))[:T].astype(bf16)
    Wq = np.asarray(Wq, dtype=np.float32)
    Wk = np.asarray(Wk, dtype=np.float32)
    Wv = np.asarray(Wv, dtype=np.float32)
    proj_w = np.asarray(proj_w, dtype=np.float32)
    proj_b = np.asarray(proj_b, dtype=np.float32)
    fc_w = np.asarray(fc_w, dtype=np.float32)
    fc_b = np.asarray(fc_b, dtype=np.float32)

    scale = np.float32(1.0 / np.sqrt(HS))
    pb8 = (proj_b / NCORES).reshape(D, 1).astype(np.float32)
    in_maps = []
    for i in range(NCORES):
        h0 = HPC * i
        wq_i = np.ascontiguousarray(
            np.concatenate([Wq[h0 + j] for j in range(HPC)], axis=1)) * scale
        wk_i = np.ascontiguousarray(
            np.concatenate([Wk[h0 + j] for j in range(HPC)], axis=1))
        wv_i = np.ascontiguousarray(
            np.concatenate([Wv[h0 + j] for j in range(HPC)], axis=1))
        pw_i = np.ascontiguousarray(
            proj_w[HPC * HS * i:HPC * HS * (i + 1), :])
        fw_i = np.ascontiguousarray(fc_w[:, VSH * i:VSH * (i + 1)])
        fb_i = np.ascontiguousarray(
            fc_b[VSH * i:VSH * (i + 1)].reshape(1, VSH)).astype(np.float32)
        in_maps.append({
            "tok": tok, "web": web, "peb": peb,
            "wqb": wq_i.astype(bf16), "wkb": wk_i.astype(bf16),
            "wvb": wv_i.astype(bf16),
            "pwb": pw_i.astype(bf16), "pb8": pb8,
            "fwb": fw_i.astype(bf16), "fbv": fb_i,
        })

    nc = _get_nc()
    trace = bool(int(os.environ.get("BASSKERNEL_TRACE", "0")))
    res = run_bass_kernel_spmd(nc, in_maps, core_ids=list(range(NCORES)),
                               trace=trace)
    if trace and res.exec_time_ns is not None:
        print(f"HW exec time: {res.exec_time_ns} ns")
        if res.instructions_and_trace is not None:
            print(f"Trace: {res.instructions_and_trace[1]}")

    out = np.empty((T, V), dtype=np.float32)
    for i in range(NCORES):
        out[:, VSH * i:VSH * (i + 1)] = res.results[i]["out"].astype(
            np.float32)
    return out
